# revision 10
# baseline (speedup 1.0000x reference)
"""AttentionBlock (B=4, C=256, H=W=64) on 8 Trainium2 NeuronCores.

Sharding: data-parallel over (batch, query-half): core i handles batch i//2,
query pixels [half*2048, (half+1)*2048), half = i%2. GroupNorm stats + k/vT
are computed per batch element (duplicated across the pair, cheap); the
O(N^2) attention work is fully sharded 8 ways. No collectives.

v3: all large matmuls run in fp8e4 (e4m3) with MatmulPerfMode.DoubleRow
(K=256 contracts in one 512-row instruction; PE streams ~216ns/matmul at
full clock). The attention inner loop is software-pipelined: the S matmuls
for key-pair p+1 are emitted before the O/R matmuls of pair p, so the PE's
in-order queue never parks a ready S behind an O waiting on ACT's exp.
exp runs on ACT in [128, 1024] chunks straight into fp8 (logits shifted -3
to fit e4m3; softmax is shift-invariant), k/v production is interleaved
into chunk 0's pair loop, per-chunk tails (1/R via reciprocal_approx_fast,
proj, residual) are injected two pairs into the next chunk. Weights are
pre-scaled by 16 on the fp8 cast (mid-range e4m3); the 1/16 rides existing
PSUM-drain ops. Host ships x in fp8 keys-rolled (own query half first),
x+rbias residual in f32, weights in bf16 (GN fold + fp8 cast on device).
"""

import numpy as np

B, C, HW = 4, 256, 4096
NH = 2048            # query pixels per core
G, CPG = 32, 8       # groups, channels per group
EPS = 1e-5
MB = HW // 128       # 32 key blocks
NP = MB // 2         # 16 key-block pairs
SW = 16.0            # fp8 weight pre-scale

_cache = {}


def build_nc():
    """Build (and cache) the Bass module."""
    if "nc" in _cache:
        return _cache["nc"]
    import concourse.tile as tile
    from concourse import bacc, mybir

    f32 = mybir.dt.float32
    bf16 = mybir.dt.bfloat16
    f8 = mybir.dt.float8e4
    AF = mybir.ActivationFunctionType
    OP = mybir.AluOpType
    PM = mybir.MatmulPerfMode

    nc = bacc.Bacc("TRN2", target_bir_lowering=False, debug=False,
                   enable_asserts=False, num_devices=8)

    # ---- DRAM I/O (host preps everything into device layout) ----
    d_xf = nc.dram_tensor("xf", [128, 2, HW], f8, kind="ExternalInput")
    d_xo = nc.dram_tensor("xo", [128, 2, NH], f32, kind="ExternalInput")
    d_w = nc.dram_tensor("w", [128, 2, 4 * C], bf16, kind="ExternalInput")
    d_sb = nc.dram_tensor("sb", [128, 2, 3], f32, kind="ExternalInput")
    d_ag = nc.dram_tensor("ag", [128, 2, G], f32, kind="ExternalInput")
    d_bg = nc.dram_tensor("bg", [G, 2, 128], f32, kind="ExternalInput")
    d_out = nc.dram_tensor("out", [128, 2, NH], f32, kind="ExternalOutput")

    with tile.TileContext(nc) as tc:
        with (
            tc.tile_pool(name="big", bufs=1) as big,
            tc.tile_pool(name="cst", bufs=1) as cst,
            tc.tile_pool(name="wrk", bufs=2) as wrk,
            tc.tile_pool(name="epool", bufs=3) as epool,
            tc.tile_pool(name="gnp", bufs=1) as gnp,
            tc.tile_pool(name="ps_s", bufs=2, space="PSUM") as ps_s,
            tc.tile_pool(name="ps_o", bufs=1, space="PSUM") as ps_o,
            tc.tile_pool(name="ps_r", bufs=1, space="PSUM") as ps_r,
            tc.tile_pool(name="ps_x", bufs=1, space="PSUM") as ps_x,
        ):
            # ---- input DMAs: x first (gates GN stats), spread over the
            # sync + gpsimd queues; params on scalar ----
            xf8 = big.tile([128, 2, HW], f8, tag="xf8")
            bstat = gnp.tile([128, 2, 8, 6], f32, tag="bstat")
            for ci in range(2):
                for j in range(8):
                    sl = slice(j * 512, (j + 1) * 512)
                    eng = nc.sync if (j % 2 == 0) else nc.gpsimd
                    eng.dma_start(out=xf8[:, ci, sl], in_=d_xf.ap()[:, ci, sl])
                    nc.vector.bn_stats(out=bstat[:, ci, j, :],
                                       in_=xf8[:, ci, sl])
            smalls = cst.tile([128, 2, 3], f32, tag="smalls")
            nc.scalar.dma_start(out=smalls, in_=d_sb.ap())
            qb = smalls[:, :, 0:1]
            kb = smalls[:, :, 1:2]
            gb = smalls[:, :, 2:3]
            ag = cst.tile([128, 2, G], f32, tag="ag")
            nc.scalar.dma_start(out=ag, in_=d_ag.ap())
            bg = cst.tile([G, 2, 128], f32, tag="bg")
            nc.scalar.dma_start(out=bg, in_=d_bg.ap())
            wall = cst.tile([128, 2, 4 * C], bf16, tag="wall")
            nc.scalar.dma_start(out=wall, in_=d_w.ap())
            xo = big.tile([128, 2, NH], f32, tag="xo")
            for ci in range(2):
                nc.gpsimd.dma_start(out=xo[:, ci, :], in_=d_xo.ap()[:, ci, :])

            epst = cst.tile([G, 1], f32, tag="epst")
            nc.vector.memset(epst, EPS)
            negc = cst.tile([128, 1], f32, tag="negc")  # softmax logit shift
            nc.vector.memset(negc, -3.0)
            # R lhsT (DoubleRow, M=128: R lands pre-broadcast on all rows)
            ones8 = cst.tile([128, 2, 128], f8, tag="ones8")
            nc.vector.memset(ones8, 1.0)
            onesr = cst.tile([1, 128], bf16, tag="onesr")  # bcast lhsT
            nc.vector.memset(onesr, 1.0)

            # ---- GroupNorm stats aggregation ----
            stats2 = gnp.tile([128, 2, 2], f32, tag="stats2")  # (mean, E[x^2])
            tmp1 = gnp.tile([128, 1], f32, tag="tmp1")
            for ci in range(2):
                nc.vector.bn_aggr(out=stats2[:, ci, :], in_=bstat[:, ci, :, :])
                nc.vector.tensor_tensor(
                    out=tmp1, in0=stats2[:, ci, 0:1], in1=stats2[:, ci, 0:1],
                    op=OP.mult)
                nc.vector.tensor_tensor(
                    out=stats2[:, ci, 1:2], in0=stats2[:, ci, 1:2], in1=tmp1,
                    op=OP.add)
            # group sums across partitions: [G, 2] = sum_ci ag[ci]^T stats2[ci]
            pg = ps_x.tile([G, 2], f32, tag="x")
            for ci in range(2):
                nc.tensor.matmul(pg, lhsT=ag[:, ci, :], rhs=stats2[:, ci, :],
                                 start=(ci == 0), stop=(ci == 1))
            # ag carries 1/CPG so pg is directly (mean_g, E[x^2]_g)
            pgs = gnp.tile([G, 2], f32, tag="pgs")
            nc.vector.tensor_copy(out=pgs, in_=pg)
            gst = gnp.tile([G, 4], f32, tag="gst")  # mean^2, var, sd, -
            nc.vector.tensor_tensor(out=gst[:, 0:1], in0=pgs[:, 0:1],
                                    in1=pgs[:, 0:1], op=OP.mult)
            nc.vector.tensor_tensor(out=gst[:, 1:2], in0=pgs[:, 1:2],
                                    in1=gst[:, 0:1], op=OP.subtract)
            gfin = gnp.tile([G, 2], f32, tag="gfin")  # (rstd_g, mean_g*rstd_g)
            nc.scalar.activation(out=gst[:, 2:3], in_=gst[:, 1:2],
                                 func=AF.Sqrt, bias=epst)
            nc.vector.reciprocal(out=gfin[:, 0:1], in_=gst[:, 2:3])
            nc.vector.tensor_tensor(out=gfin[:, 1:2], in0=pgs[:, 0:1],
                                    in1=gfin[:, 0:1], op=OP.mult)
            # bg carries gn_w, so pbc = (scale_c, mean_c*scale_c);
            # bias_c = gn_b - mean_c*scale_c
            scbc = gnp.tile([128, 2, 2], f32, tag="scbc")
            for ci in range(2):
                pbc = ps_x.tile([128, 2], f32, tag="x")
                nc.tensor.matmul(pbc, lhsT=bg[:, ci, :], rhs=gfin,
                                 start=True, stop=True)
                nc.vector.tensor_copy(out=scbc[:, ci, 0:1], in_=pbc[:, 0:1])
                nc.vector.tensor_tensor(out=scbc[:, ci, 1:2], in0=gb[:, ci, :],
                                        in1=pbc[:, 1:2], op=OP.subtract)

            # ---- fp8 weights first (gates k/q/v production):
            # W8 = W * scale_c * 16 (q,k,v), W * 16 (proj)
            w8 = cst.tile([128, 2, 4 * C], f8, tag="w8")
            for ci in range(2):
                nc.vector.tensor_scalar(
                    out=w8[:, ci, 0:3 * C], in0=wall[:, ci, 0:3 * C],
                    scalar1=scbc[:, ci, 0:1], scalar2=SW,
                    op0=OP.mult, op1=OP.mult)
                nc.vector.tensor_scalar(
                    out=w8[:, ci, 3 * C:4 * C], in0=wall[:, ci, 3 * C:4 * C],
                    scalar1=SW, scalar2=None, op0=OP.mult)

            # ---- channel biases from the GN fold (on unscaled bf16 W) ----
            bcr = gnp.tile([128, 2, 2], bf16, tag="bcr")  # bias_c, 2 copies
            for ci in range(2):
                nc.vector.tensor_copy(out=bcr[:, ci, 0:1],
                                      in_=scbc[:, ci, 1:2])
                nc.vector.tensor_copy(out=bcr[:, ci, 1:2],
                                      in_=scbc[:, ci, 1:2])
            # bias2[:, cb, wi] = qb/kb + W_wi @ bias_c
            bias2 = gnp.tile([128, 2, 2], f32, tag="bias2")
            for wi in range(2):
                for cb in range(2):
                    pbias = ps_x.tile([128, 2], f32, tag="x")
                    for ci in range(2):
                        nc.tensor.matmul(
                            pbias,
                            lhsT=wall[:, ci, wi * C + cb * 128:
                                      wi * C + (cb + 1) * 128],
                            rhs=bcr[:, ci, :], start=(ci == 0), stop=(ci == 1))
                    nc.vector.tensor_tensor(
                        out=bias2[:, cb, wi:wi + 1], in0=pbias[:, 0:1],
                        in1=(qb if wi == 0 else kb)[:, cb, :], op=OP.add)
            # v bias along FREE dim: vb2[1, c_out] = bias_c^T Wv
            pvb = ps_x.tile([1, 512], f32, tag="x")
            for ci in range(2):
                nc.tensor.matmul(pvb[:, 0:C], lhsT=bcr[:, ci, 0:1],
                                 rhs=wall[:, ci, 2 * C:3 * C],
                                 start=(ci == 0), stop=(ci == 1))
            vb2r = gnp.tile([1, C], bf16, tag="vb2r")
            nc.scalar.copy(out=vb2r, in_=pvb[:, 0:C])
            pvbb = ps_x.tile([128, 512], f32, tag="x")
            nc.tensor.matmul(pvbb[:, 0:C], lhsT=onesr, rhs=vb2r,
                             start=True, stop=True)
            vb2b = gnp.tile([128, 2, C], f32, tag="vb2b")
            nc.vector.tensor_copy(out=vb2b[:, 0, :], in_=pvbb[:, 0:C])
            nc.vector.tensor_copy(out=vb2b[:, 1, :], in_=pvbb[:, 0:C])

            kt = big.tile([128, 2, HW], f8, tag="kt")
            qt = big.tile([128, 2, NH], f8, tag="qt")
            vT = big.tile([128, MB, C], f8, tag="vT")

            def qt_unit(j):  # 512 queries
                sl = slice(j * 512, (j + 1) * 512)
                pq = ps_s.tile([128, 2, 512], f32, tag="s", name=f"pq{j}")
                for cb in range(2):
                    nc.tensor.matmul(
                        pq[:, cb, :],
                        lhsT=w8[:, :, cb * 128:(cb + 1) * 128],
                        rhs=xf8[:, :, sl], start=True, stop=True,
                        perf_mode=PM.DoubleRow)
                for cb in range(2):
                    nc.vector.tensor_scalar(
                        out=qt[:, cb, sl], in0=pq[:, cb, :],
                        scalar1=1.0 / SW, scalar2=bias2[:, cb, 0:1],
                        op0=OP.mult, op1=OP.add)

            def kt_unit(u):  # 512 pixels
                sl = slice(u * 512, (u + 1) * 512)
                pk = ps_s.tile([128, 2, 512], f32, tag="s", name=f"pk{u}")
                for cb in range(2):
                    nc.tensor.matmul(
                        pk[:, cb, :],
                        lhsT=w8[:, :, C + cb * 128:C + (cb + 1) * 128],
                        rhs=xf8[:, :, sl], start=True, stop=True,
                        perf_mode=PM.DoubleRow)
                for cb in range(2):
                    nc.vector.tensor_scalar(
                        out=kt[:, cb, sl], in0=pk[:, cb, :],
                        scalar1=1.0 / SW, scalar2=bias2[:, cb, 1:2],
                        op0=OP.mult, op1=OP.add)

            def vt_unit(u):  # 4 key blocks = 2 pairs
                pv = ps_s.tile([128, 2, 512], f32, tag="s", name=f"pv{u}")
                for h in range(2):
                    for par in range(2):
                        mb = 4 * u + 2 * h + par
                        msl = slice(mb * 128, (mb + 1) * 128)
                        nc.tensor.matmul(
                            pv[:, par, h * 256:h * 256 + 256],
                            lhsT=xf8[:, :, msl],
                            rhs=w8[:, :, 2 * C:3 * C], start=True, stop=True,
                            perf_mode=PM.DoubleRow)
                for h in range(2):
                    p = 2 * u + h
                    nc.vector.scalar_tensor_tensor(
                        out=vT[:, 2 * p:2 * p + 2, :],
                        in0=pv[:, :, h * 256:h * 256 + 256],
                        scalar=1.0 / SW, in1=vb2b,
                        op0=OP.mult, op1=OP.add)

            state = {}

            def s_pair(j, p):
                sl = slice(j * 512, (j + 1) * 512)
                sp = ps_s.tile([128, 2, 512], f32, tag="s", name=f"sp{j}_{p}")
                for par in range(2):
                    mb = 2 * p + par
                    nc.tensor.matmul(
                        sp[:, par, :],
                        lhsT=kt[:, :, mb * 128:(mb + 1) * 128],
                        rhs=qt[:, :, sl], start=True, stop=True,
                        perf_mode=PM.DoubleRow)
                return sp

            def o_r(j, p, et):
                po, pr = state["po"], state["pr"]
                for cb in range(2):
                    nc.tensor.matmul(
                        po[:, cb, :],
                        lhsT=vT[:, 2 * p:2 * p + 2, cb * 128:(cb + 1) * 128],
                        rhs=et, start=(p == 0), stop=(p == NP - 1),
                        perf_mode=PM.DoubleRow, skip_group_check=True)
                nc.tensor.matmul(
                    pr, lhsT=ones8, rhs=et,
                    start=(p == 0), stop=(p == NP - 1),
                    perf_mode=PM.DoubleRow, skip_group_check=True)

            def attn_chunk(j, produce=False, inject=None):
                state["po"] = ps_o.tile([128, 2, 512], f32, tag="o",
                                        name=f"po{j}")
                state["pr"] = ps_r.tile([128, 512], f32, tag="r",
                                        name=f"pr{j}")
                prev_sp = prev_et = None
                for p in range(NP):
                    if produce and p % 2 == 0:
                        kt_unit(p // 2)
                        vt_unit(p // 2)
                    sp = s_pair(j, p)
                    if prev_et is not None:
                        o_r(j, p - 1, prev_et)
                    if inject is not None and p == 2:
                        tail_proj(inject)
                    # logit shift keeps exp in e4m3 range (max logit ~8:
                    # e^(8-3)=148 < 240); softmax is shift-invariant
                    et = epool.tile([128, 2, 512], f8, tag="et")
                    nc.scalar.activation(out=et, in_=sp, func=AF.Exp,
                                         scale=1.0 / SW, bias=negc)
                    prev_sp, prev_et = sp, et
                o_r(j, NP - 1, prev_et)

            def tail_norm(j):
                """Free po/pr: copy O out, build 1/R."""
                po, pr = state["po"], state["pr"]
                onorm = wrk.tile([128, 2, 512], f32, tag="onorm")
                nc.vector.tensor_copy(out=onorm, in_=po)  # frees po
                rb = wrk.tile([128, 512], f32, tag="rb")
                nc.vector.reciprocal_approx_fast(out=rb, in_=pr)  # frees pr
                onormed = wrk.tile([128, 2, 512], f8, tag="onormed")
                for cb in range(2):
                    nc.vector.tensor_tensor(
                        out=onormed[:, cb, :], in0=onorm[:, cb, :], in1=rb,
                        op=OP.mult)
                state[f"on{j}"] = onormed

            def tail_proj(j):
                """Project, add residual base, store chunk j."""
                sl = slice(j * 512, (j + 1) * 512)
                onormed = state.pop(f"on{j}")
                for co in range(2):
                    pp = ps_x.tile([128, 512], f32, tag="x", name=f"pp{j}_{co}")
                    nc.tensor.matmul(
                        pp, lhsT=w8[:, :, 3 * C + co * 128:
                                    3 * C + (co + 1) * 128],
                        rhs=onormed, start=True, stop=True,
                        perf_mode=PM.DoubleRow)
                    outt = wrk.tile([128, 512], f32, tag="outt")
                    nc.vector.scalar_tensor_tensor(
                        out=outt, in0=pp, scalar=1.0 / SW,
                        in1=xo[:, co, sl], op0=OP.mult, op1=OP.add)
                    nc.sync.dma_start(out=d_out.ap()[:, co, sl], in_=outt)

            # ---- chunk 0: interleave k/v production with attention ----
            qt_unit(0)
            attn_chunk(0, produce=True)
            for j in range(1, 4):
                qt_unit(j)
            tail_norm(0)
            for j in range(1, 4):
                attn_chunk(j, inject=j - 1)
                tail_norm(j)
            tail_proj(3)

    nc.compile()
    _cache["nc"] = nc
    return nc


def _prep_maps(x, gn_w, gn_b, qkv_w, qkv_b, proj_w, proj_b):
    """Host-side sharding + layout prep. Returns list of 8 in_maps."""
    import ml_dtypes
    f8 = ml_dtypes.float8_e4m3
    bf16 = ml_dtypes.bfloat16

    x = np.asarray(x, np.float32)
    qkv_w = np.asarray(qkv_w, np.float32)
    qkv_b = np.asarray(qkv_b, np.float32)
    proj_w = np.asarray(proj_w, np.float32)
    proj_b = np.asarray(proj_b, np.float32)
    gn_w = np.asarray(gn_w, np.float32)
    gn_b = np.asarray(gn_b, np.float32)

    def chunked(a):  # [256, ...] -> [128, 2, ...]
        return np.ascontiguousarray(a.reshape(2, 128, *a.shape[1:]).transpose(
            1, 0, *range(2, a.ndim + 1)))

    wq = qkv_w[0:C].T
    wk = qkv_w[C:2 * C].T
    wv = qkv_w[2 * C:3 * C].T
    wp = proj_w.T
    wall = chunked(np.concatenate([wq, wk, wv, wp], axis=1)).astype(bf16)
    rbias = proj_w @ qkv_b[2 * C:3 * C] + proj_b   # v-bias fold + proj bias
    smalls = chunked(np.stack([qkv_b[0:C], qkv_b[C:2 * C], gn_b], axis=1))

    cidx = np.arange(C)
    ag_full = (cidx[:, None] // CPG == np.arange(G)[None, :]).astype(np.float32)
    ag = chunked(ag_full / CPG)                     # [128, 2, G], carries 1/8
    bg_full = ag_full * gn_w[:, None]               # carries gn_w
    bg = np.ascontiguousarray(
        bg_full.reshape(2, 128, G).transpose(2, 0, 1))  # [G, 2, 128]

    maps = []
    for core in range(8):
        b, half = core // 2, core % 2
        xflat = x[b].reshape(C, HW)
        own = xflat[:, half * NH:(half + 1) * NH]
        other = xflat[:, (1 - half) * NH:(2 - half) * NH]
        xroll = np.concatenate([own, other], axis=1)  # own queries first
        xo = own + rbias[:, None]
        maps.append({
            "xf": chunked(xroll).astype(f8),
            "xo": chunked(xo),
            "w": wall, "sb": smalls, "ag": ag, "bg": bg,
        })
    return maps


def kernel(x, gn_w, gn_b, qkv_w, qkv_b, proj_w, proj_b):
    import concourse.bass_utils as bu
    nc = build_nc()
    maps = _prep_maps(x, gn_w, gn_b, qkv_w, qkv_b, proj_w, proj_b)
    res = bu.run_bass_kernel_spmd(nc, maps, core_ids=list(range(8)))
    out = np.empty((B, C, HW), np.float32)
    for core in range(8):
        b, half = core // 2, core % 2
        o = res.results[core]["out"]                # [128, 2, NH]
        out[b, :, half * NH:(half + 1) * NH] = \
            o.transpose(1, 0, 2).reshape(C, NH)
    return out.reshape(B, C, 64, 64)


# revision 17
# speedup vs baseline: 1.1903x; 1.1903x over previous
"""AttentionBlock (B=4, C=256, H=W=64) on 8 Trainium2 NeuronCores.

Sharding: data-parallel over (batch, query-half): core i handles batch i//2,
query pixels [half*2048, (half+1)*2048), half = i%2. GroupNorm stats + k/vT
are computed per batch element (duplicated across the pair, cheap); the
O(N^2) attention work is fully sharded 8 ways. No collectives.

v3: all large matmuls run in fp8e4 (e4m3) with MatmulPerfMode.DoubleRow
(K=256 contracts in one 512-row instruction; PE streams ~216ns/matmul at
full clock). The attention inner loop is software-pipelined: the S matmuls
for key-pair p+1 are emitted before the O/R matmuls of pair p, so the PE's
in-order queue never parks a ready S behind an O waiting on ACT's exp.
exp runs on ACT in [128, 1024] chunks straight into fp8 (logits shifted -3
to fit e4m3; softmax is shift-invariant), k/v production is interleaved
into chunk 0's pair loop, per-chunk tails (1/R via reciprocal_approx_fast,
proj, residual) are injected two pairs into the next chunk. Weights are
pre-scaled by 16 on the fp8 cast (mid-range e4m3); the 1/16 rides existing
PSUM-drain ops. Host ships x in fp8 keys-rolled (own query half first),
x+rbias residual in f32, weights in bf16 (GN fold + fp8 cast on device).
"""

import numpy as np

B, C, HW = 4, 256, 4096
NH = 2048            # query pixels per core
G, CPG = 32, 8       # groups, channels per group
EPS = 1e-5
MB = HW // 128       # 32 key blocks
NP = MB // 2         # 16 key-block pairs
SW = 16.0            # fp8 weight pre-scale

_cache = {}


def build_nc():
    """Build (and cache) the Bass module."""
    if "nc" in _cache:
        return _cache["nc"]
    import concourse.tile as tile
    from concourse import bacc, mybir

    f32 = mybir.dt.float32
    bf16 = mybir.dt.bfloat16
    f8 = mybir.dt.float8e4
    AF = mybir.ActivationFunctionType
    OP = mybir.AluOpType
    PM = mybir.MatmulPerfMode

    nc = bacc.Bacc("TRN2", target_bir_lowering=False, debug=False,
                   enable_asserts=False, num_devices=8)

    # ---- DRAM I/O (host preps everything into device layout) ----
    d_xf = nc.dram_tensor("xf", [128, 2, HW], f8, kind="ExternalInput")
    d_xo = nc.dram_tensor("xo", [128, 2, NH], f32, kind="ExternalInput")
    d_w = nc.dram_tensor("w", [128, 2, 4 * C], bf16, kind="ExternalInput")
    d_sb = nc.dram_tensor("sb", [128, 2, 3], f32, kind="ExternalInput")
    d_ag = nc.dram_tensor("ag", [128, 2, G], f32, kind="ExternalInput")
    d_bg = nc.dram_tensor("bg", [G, 2, 128], f32, kind="ExternalInput")
    d_out = nc.dram_tensor("out", [128, 2, NH], f32, kind="ExternalOutput")

    with tile.TileContext(nc) as tc:
        with (
            tc.tile_pool(name="big", bufs=1) as big,
            tc.tile_pool(name="cst", bufs=1) as cst,
            tc.tile_pool(name="wrk", bufs=2) as wrk,
            tc.tile_pool(name="epool", bufs=3) as epool,
            tc.tile_pool(name="gnp", bufs=1) as gnp,
            tc.tile_pool(name="ps_s", bufs=2, space="PSUM") as ps_s,
            tc.tile_pool(name="ps_o", bufs=1, space="PSUM") as ps_o,
            tc.tile_pool(name="ps_r", bufs=1, space="PSUM") as ps_r,
            tc.tile_pool(name="ps_x", bufs=1, space="PSUM") as ps_x,
        ):
            # ---- input DMAs: x first (gates GN stats), spread over the
            # sync + gpsimd queues; params on scalar ----
            xf8 = big.tile([128, 2, HW], f8, tag="xf8")
            bstat = gnp.tile([128, 2, 8, 6], f32, tag="bstat")
            for ci in range(2):
                for j in range(8):
                    sl = slice(j * 512, (j + 1) * 512)
                    eng = nc.sync if (j % 2 == 0) else nc.scalar
                    eng.dma_start(out=xf8[:, ci, sl], in_=d_xf.ap()[:, ci, sl])
                    nc.vector.bn_stats(out=bstat[:, ci, j, :],
                                       in_=xf8[:, ci, sl])
            smalls = cst.tile([128, 2, 3], f32, tag="smalls")
            nc.scalar.dma_start(out=smalls, in_=d_sb.ap())
            qb = smalls[:, :, 0:1]
            kb = smalls[:, :, 1:2]
            gb = smalls[:, :, 2:3]
            ag = cst.tile([128, 2, G], f32, tag="ag")
            nc.scalar.dma_start(out=ag, in_=d_ag.ap())
            bg = cst.tile([G, 2, 128], f32, tag="bg")
            nc.scalar.dma_start(out=bg, in_=d_bg.ap())
            wall = cst.tile([128, 2, 4 * C], bf16, tag="wall")
            nc.scalar.dma_start(out=wall, in_=d_w.ap())
            xo = big.tile([128, 2, NH], f32, tag="xo")
            for ci in range(2):
                for j in range(2):
                    sl = slice(j * 1024, (j + 1) * 1024)
                    nc.sync.dma_start(out=xo[:, ci, sl],
                                      in_=d_xo.ap()[:, ci, sl])

            epst = cst.tile([G, 1], f32, tag="epst")
            nc.vector.memset(epst, EPS)
            negc = cst.tile([128, 1], f32, tag="negc")  # softmax logit shift
            nc.vector.memset(negc, -3.0)
            # R lhsT (DoubleRow, M=128: R lands pre-broadcast on all rows)
            ones8 = cst.tile([128, 2, 128], f8, tag="ones8")
            nc.vector.memset(ones8, 1.0)
            onesr = cst.tile([1, 128], bf16, tag="onesr")  # bcast lhsT
            nc.vector.memset(onesr, 1.0)

            # ---- GroupNorm stats aggregation ----
            stats2 = gnp.tile([128, 2, 2], f32, tag="stats2")  # (mean, E[x^2])
            tmp1 = gnp.tile([128, 1], f32, tag="tmp1")
            for ci in range(2):
                nc.vector.bn_aggr(out=stats2[:, ci, :], in_=bstat[:, ci, :, :])
                nc.vector.tensor_tensor(
                    out=tmp1, in0=stats2[:, ci, 0:1], in1=stats2[:, ci, 0:1],
                    op=OP.mult)
                nc.vector.tensor_tensor(
                    out=stats2[:, ci, 1:2], in0=stats2[:, ci, 1:2], in1=tmp1,
                    op=OP.add)
            # group sums across partitions: [G, 2] = sum_ci ag[ci]^T stats2[ci]
            pg = ps_x.tile([G, 2], f32, tag="x")
            for ci in range(2):
                nc.tensor.matmul(pg, lhsT=ag[:, ci, :], rhs=stats2[:, ci, :],
                                 start=(ci == 0), stop=(ci == 1))
            # ag carries 1/CPG so pg is directly (mean_g, E[x^2]_g)
            pgs = gnp.tile([G, 2], f32, tag="pgs")
            nc.vector.tensor_copy(out=pgs, in_=pg)
            gst = gnp.tile([G, 4], f32, tag="gst")  # mean^2, var, sd, -
            nc.vector.tensor_tensor(out=gst[:, 0:1], in0=pgs[:, 0:1],
                                    in1=pgs[:, 0:1], op=OP.mult)
            nc.vector.tensor_tensor(out=gst[:, 1:2], in0=pgs[:, 1:2],
                                    in1=gst[:, 0:1], op=OP.subtract)
            gfin = gnp.tile([G, 2], f32, tag="gfin")  # (rstd_g, mean_g*rstd_g)
            nc.scalar.activation(out=gst[:, 2:3], in_=gst[:, 1:2],
                                 func=AF.Sqrt, bias=epst)
            nc.vector.reciprocal(out=gfin[:, 0:1], in_=gst[:, 2:3])
            nc.vector.tensor_tensor(out=gfin[:, 1:2], in0=pgs[:, 0:1],
                                    in1=gfin[:, 0:1], op=OP.mult)
            # bg carries gn_w, so pbc = (scale_c, mean_c*scale_c);
            # bias_c = gn_b - mean_c*scale_c
            scbc = gnp.tile([128, 2, 2], f32, tag="scbc")
            for ci in range(2):
                pbc = ps_x.tile([128, 2], f32, tag="x")
                nc.tensor.matmul(pbc, lhsT=bg[:, ci, :], rhs=gfin,
                                 start=True, stop=True)
                nc.vector.tensor_copy(out=scbc[:, ci, 0:1], in_=pbc[:, 0:1])
                nc.vector.tensor_tensor(out=scbc[:, ci, 1:2], in0=gb[:, ci, :],
                                        in1=pbc[:, 1:2], op=OP.subtract)

            # ---- fp8 weights first (gates k/q/v production):
            # W8 = W * scale_c * 16 (q,k,v), W * 16 (proj)
            w8 = cst.tile([128, 2, 4 * C], f8, tag="w8")
            for ci in range(2):
                nc.vector.tensor_scalar(
                    out=w8[:, ci, 0:3 * C], in0=wall[:, ci, 0:3 * C],
                    scalar1=scbc[:, ci, 0:1], scalar2=SW,
                    op0=OP.mult, op1=OP.mult)
                nc.vector.tensor_scalar(
                    out=w8[:, ci, 3 * C:4 * C], in0=wall[:, ci, 3 * C:4 * C],
                    scalar1=SW, scalar2=None, op0=OP.mult)

            # ---- channel biases from the GN fold (on unscaled bf16 W) ----
            bcr = gnp.tile([128, 2, 2], bf16, tag="bcr")  # bias_c, 2 copies
            for ci in range(2):
                nc.vector.tensor_copy(out=bcr[:, ci, 0:1],
                                      in_=scbc[:, ci, 1:2])
                nc.vector.tensor_copy(out=bcr[:, ci, 1:2],
                                      in_=scbc[:, ci, 1:2])
            # q bias: bias2[:, cb] = qb + Wq @ bias_c. (The k-side bias adds a
            # per-query constant to every logit -- softmax-invariant -- so k
            # needs no bias at all.)
            bias2 = gnp.tile([128, 2, 1], f32, tag="bias2")
            for cb in range(2):
                pbias = ps_x.tile([128, 2], f32, tag="x")
                for ci in range(2):
                    nc.tensor.matmul(
                        pbias,
                        lhsT=wall[:, ci, cb * 128:(cb + 1) * 128],
                        rhs=bcr[:, ci, :], start=(ci == 0), stop=(ci == 1))
                nc.vector.tensor_tensor(
                    out=bias2[:, cb, 0:1], in0=pbias[:, 0:1],
                    in1=qb[:, cb, :], op=OP.add)
            # v bias along FREE dim: vb2[1, c_out] = bias_c^T Wv
            pvb = ps_x.tile([1, 512], f32, tag="x")
            for ci in range(2):
                nc.tensor.matmul(pvb[:, 0:C], lhsT=bcr[:, ci, 0:1],
                                 rhs=wall[:, ci, 2 * C:3 * C],
                                 start=(ci == 0), stop=(ci == 1))
            vb2r = gnp.tile([1, C], bf16, tag="vb2r")
            nc.scalar.copy(out=vb2r, in_=pvb[:, 0:C])
            pvbb = ps_x.tile([128, 512], f32, tag="x")
            nc.tensor.matmul(pvbb[:, 0:C], lhsT=onesr, rhs=vb2r,
                             start=True, stop=True)
            vb2b = gnp.tile([128, 2, C], f32, tag="vb2b")
            nc.vector.tensor_copy(out=vb2b[:, 0, :], in_=pvbb[:, 0:C])
            nc.vector.tensor_copy(out=vb2b[:, 1, :], in_=pvbb[:, 0:C])

            kt = big.tile([128, 2, HW], f8, tag="kt")
            qt = big.tile([128, 2, NH], f8, tag="qt")
            vT = big.tile([128, MB, C], f8, tag="vT")

            def qt_unit(j):  # 512 queries
                sl = slice(j * 512, (j + 1) * 512)
                pq = ps_s.tile([128, 2, 512], f32, tag="s", name=f"pq{j}")
                for cb in range(2):
                    nc.tensor.matmul(
                        pq[:, cb, :],
                        lhsT=w8[:, :, cb * 128:(cb + 1) * 128],
                        rhs=xf8[:, :, sl], start=True, stop=True,
                        perf_mode=PM.DoubleRow)
                for cb in range(2):
                    nc.vector.tensor_scalar(
                        out=qt[:, cb, sl], in0=pq[:, cb, :],
                        scalar1=1.0 / SW, scalar2=bias2[:, cb, :],
                        op0=OP.mult, op1=OP.add)

            def kt_unit(u):  # 512 pixels
                sl = slice(u * 512, (u + 1) * 512)
                pk = ps_s.tile([128, 2, 512], f32, tag="s", name=f"pk{u}")
                for cb in range(2):
                    nc.tensor.matmul(
                        pk[:, cb, :],
                        lhsT=w8[:, :, C + cb * 128:C + (cb + 1) * 128],
                        rhs=xf8[:, :, sl], start=True, stop=True,
                        perf_mode=PM.DoubleRow)
                nc.vector.tensor_scalar(
                    out=kt[:, :, sl], in0=pk, scalar1=1.0 / SW, scalar2=None,
                    op0=OP.mult)

            def vt_unit(u):  # 4 key blocks = 2 pairs
                pv = ps_s.tile([128, 2, 512], f32, tag="s", name=f"pv{u}")
                for h in range(2):
                    for par in range(2):
                        mb = 4 * u + 2 * h + par
                        msl = slice(mb * 128, (mb + 1) * 128)
                        nc.tensor.matmul(
                            pv[:, par, h * 256:h * 256 + 256],
                            lhsT=xf8[:, :, msl],
                            rhs=w8[:, :, 2 * C:3 * C], start=True, stop=True,
                            perf_mode=PM.DoubleRow)
                for h in range(2):
                    p = 2 * u + h
                    nc.vector.scalar_tensor_tensor(
                        out=vT[:, 2 * p:2 * p + 2, :],
                        in0=pv[:, :, h * 256:h * 256 + 256],
                        scalar=1.0 / SW, in1=vb2b,
                        op0=OP.mult, op1=OP.add)

            state = {}

            def s_pair(j, p):
                sl = slice(j * 512, (j + 1) * 512)
                sp = ps_s.tile([128, 2, 512], f32, tag="s", name=f"sp{j}_{p}")
                for par in range(2):
                    mb = 2 * p + par
                    nc.tensor.matmul(
                        sp[:, par, :],
                        lhsT=kt[:, :, mb * 128:(mb + 1) * 128],
                        rhs=qt[:, :, sl], start=True, stop=True,
                        perf_mode=PM.DoubleRow)
                return sp

            def o_r(j, p, et):
                po, pr = state["po"], state["pr"]
                for cb in range(2):
                    nc.tensor.matmul(
                        po[:, cb, :],
                        lhsT=vT[:, 2 * p:2 * p + 2, cb * 128:(cb + 1) * 128],
                        rhs=et, start=(p == 0), stop=(p == NP - 1),
                        perf_mode=PM.DoubleRow, skip_group_check=True)
                nc.tensor.matmul(
                    pr, lhsT=ones8, rhs=et,
                    start=(p == 0), stop=(p == NP - 1),
                    perf_mode=PM.DoubleRow, skip_group_check=True)

            def attn_chunk(j, produce=False, inject=None):
                state["po"] = ps_o.tile([128, 2, 512], f32, tag="o",
                                        name=f"po{j}")
                state["pr"] = ps_r.tile([128, 512], f32, tag="r",
                                        name=f"pr{j}")
                prev_sp = prev_et = None
                for p in range(NP):
                    if produce and p % 2 == 0 and p // 2 + 1 < 8:
                        kt_unit(p // 2 + 1)
                        vt_unit(p // 2 + 1)
                    sp = s_pair(j, p)
                    if prev_et is not None:
                        o_r(j, p - 1, prev_et)
                    if inject is not None and p == 2:
                        tail_proj(inject)
                    # logit shift keeps exp in e4m3 range (max logit ~8:
                    # e^(8-3)=148 < 240); softmax is shift-invariant
                    et = epool.tile([128, 2, 512], f8, tag="et")
                    nc.scalar.activation(out=et, in_=sp, func=AF.Exp,
                                         scale=1.0 / SW, bias=negc)
                    prev_sp, prev_et = sp, et
                o_r(j, NP - 1, prev_et)

            def tail_norm(j):
                """Free po/pr: copy O out, build 1/R."""
                po, pr = state["po"], state["pr"]
                onorm = wrk.tile([128, 2, 512], f32, tag="onorm")
                nc.vector.tensor_copy(out=onorm, in_=po)  # frees po
                rb = wrk.tile([128, 512], f32, tag="rb")
                nc.vector.reciprocal_approx_fast(out=rb, in_=pr)  # frees pr
                onormed = wrk.tile([128, 2, 512], f8, tag="onormed")
                for cb in range(2):
                    nc.vector.tensor_tensor(
                        out=onormed[:, cb, :], in0=onorm[:, cb, :], in1=rb,
                        op=OP.mult)
                state[f"on{j}"] = onormed

            def tail_proj(j):
                """Project, add residual base, store chunk j."""
                sl = slice(j * 512, (j + 1) * 512)
                onormed = state.pop(f"on{j}")
                for co in range(2):
                    pp = ps_x.tile([128, 512], f32, tag="x", name=f"pp{j}_{co}")
                    nc.tensor.matmul(
                        pp, lhsT=w8[:, :, 3 * C + co * 128:
                                    3 * C + (co + 1) * 128],
                        rhs=onormed, start=True, stop=True,
                        perf_mode=PM.DoubleRow)
                    outt = wrk.tile([128, 512], f32, tag="outt")
                    nc.vector.scalar_tensor_tensor(
                        out=outt, in0=pp, scalar=1.0 / SW,
                        in1=xo[:, co, sl], op0=OP.mult, op1=OP.add)
                    nc.sync.dma_start(out=d_out.ap()[:, co, sl], in_=outt)

            # ---- chunk 0: interleave k/v production with attention,
            # producing one 512-pixel unit ahead of the consuming pairs ----
            qt_unit(0)
            kt_unit(0)
            vt_unit(0)
            attn_chunk(0, produce=True)
            for j in range(1, 4):
                qt_unit(j)
            tail_norm(0)
            for j in range(1, 4):
                attn_chunk(j, inject=j - 1)
                tail_norm(j)
            tail_proj(3)

    nc.compile()
    _cache["nc"] = nc
    return nc


def _prep_maps(x, gn_w, gn_b, qkv_w, qkv_b, proj_w, proj_b):
    """Host-side sharding + layout prep. Returns list of 8 in_maps."""
    import ml_dtypes
    f8 = ml_dtypes.float8_e4m3
    bf16 = ml_dtypes.bfloat16

    x = np.asarray(x, np.float32)
    qkv_w = np.asarray(qkv_w, np.float32)
    qkv_b = np.asarray(qkv_b, np.float32)
    proj_w = np.asarray(proj_w, np.float32)
    proj_b = np.asarray(proj_b, np.float32)
    gn_w = np.asarray(gn_w, np.float32)
    gn_b = np.asarray(gn_b, np.float32)

    def chunked(a):  # [256, ...] -> [128, 2, ...]
        return np.ascontiguousarray(a.reshape(2, 128, *a.shape[1:]).transpose(
            1, 0, *range(2, a.ndim + 1)))

    wq = qkv_w[0:C].T
    wk = qkv_w[C:2 * C].T
    wv = qkv_w[2 * C:3 * C].T
    wp = proj_w.T
    wall = chunked(np.concatenate([wq, wk, wv, wp], axis=1)).astype(bf16)
    rbias = proj_w @ qkv_b[2 * C:3 * C] + proj_b   # v-bias fold + proj bias
    smalls = chunked(np.stack([qkv_b[0:C], qkv_b[C:2 * C], gn_b], axis=1))

    cidx = np.arange(C)
    ag_full = (cidx[:, None] // CPG == np.arange(G)[None, :]).astype(np.float32)
    ag = chunked(ag_full / CPG)                     # [128, 2, G], carries 1/8
    bg_full = ag_full * gn_w[:, None]               # carries gn_w
    bg = np.ascontiguousarray(
        bg_full.reshape(2, 128, G).transpose(2, 0, 1))  # [G, 2, 128]

    maps = []
    for core in range(8):
        b, half = core // 2, core % 2
        xflat = x[b].reshape(C, HW)
        own = xflat[:, half * NH:(half + 1) * NH]
        other = xflat[:, (1 - half) * NH:(2 - half) * NH]
        xroll = np.concatenate([own, other], axis=1)  # own queries first
        xo = own + rbias[:, None]
        maps.append({
            "xf": chunked(xroll).astype(f8),
            "xo": chunked(xo),
            "w": wall, "sb": smalls, "ag": ag, "bg": bg,
        })
    return maps


def kernel(x, gn_w, gn_b, qkv_w, qkv_b, proj_w, proj_b):
    import concourse.bass_utils as bu
    nc = build_nc()
    maps = _prep_maps(x, gn_w, gn_b, qkv_w, qkv_b, proj_w, proj_b)
    res = bu.run_bass_kernel_spmd(nc, maps, core_ids=list(range(8)))
    out = np.empty((B, C, HW), np.float32)
    for core in range(8):
        b, half = core // 2, core % 2
        o = res.results[core]["out"]                # [128, 2, NH]
        out[b, :, half * NH:(half + 1) * NH] = \
            o.transpose(1, 0, 2).reshape(C, NH)
    return out.reshape(B, C, 64, 64)


# revision 21
# speedup vs baseline: 1.3100x; 1.1005x over previous
"""AttentionBlock (B=4, C=256, H=W=64) on 8 Trainium2 NeuronCores.

Sharding: data-parallel over (batch, query-half): core i handles batch i//2,
query pixels [half*2048, (half+1)*2048), half = i%2. GroupNorm stats are
computed per batch element (duplicated across the pair, cheap); the O(N^2)
attention work is fully sharded 8 ways. No collectives.

v5: all large matmuls are fp8e4 DoubleRow (K=256 per 512-row instruction).
k and v are never materialized -- by associativity the attention runs on
raw fp8 x from both ends:
    S  = k^T q       = x^T (Wk^T q)           (qk made once per chunk)
    O  = v E         = Wv (x E) + bv R        (xE accumulated in PSUM)
so the inner loop is just S(pair) -> exp -> xE/R accumulate, identical for
all 64 pairs, with zero per-pair weight/bias traffic. The GN channel scale
rides the per-partition qk/q drains, bv/bq fold into per-partition drain
biases, and the k-side bias is dropped outright (a per-query logit shift is
softmax-invariant). exp runs on ACT in [128,1024] chunks into fp8 with a -3
logit shift (e4m3 range), software-pipelined one S-pair ahead of the xE/R
consumers. 1/R uses reciprocal_approx_fast on the PE-broadcast R block.
Host ships x twice in fp8 ([cin, pix] and [pix, cin], query half first),
the residual base x+rbias in f32, and weights in bf16 (GN fold + 16x fp8
cast on device; Wk^T additionally raw fp8).
"""

import numpy as np

B, C, HW = 4, 256, 4096
NH = 2048            # query pixels per core
G, CPG = 32, 8       # groups, channels per group
EPS = 1e-5
MB = HW // 128       # 32 key blocks
NP = MB // 2         # 16 key-block pairs
SW = 16.0            # fp8 weight pre-scale

_cache = {}


def build_nc():
    """Build (and cache) the Bass module."""
    if "nc" in _cache:
        return _cache["nc"]
    import concourse.tile as tile
    from concourse import bacc, mybir

    f32 = mybir.dt.float32
    bf16 = mybir.dt.bfloat16
    f8 = mybir.dt.float8e4
    AF = mybir.ActivationFunctionType
    OP = mybir.AluOpType
    PM = mybir.MatmulPerfMode

    nc = bacc.Bacc("TRN2", target_bir_lowering=False, debug=False,
                   enable_asserts=False, num_devices=8)

    # ---- DRAM I/O (host preps everything into device layout) ----
    d_xf = nc.dram_tensor("xf", [128, 2, HW], f8, kind="ExternalInput")
    d_xt = nc.dram_tensor("xt", [128, MB, C], f8, kind="ExternalInput")
    d_xo = nc.dram_tensor("xo", [128, 2, NH], f32, kind="ExternalInput")
    d_w = nc.dram_tensor("w", [128, 2, 4 * C], bf16, kind="ExternalInput")
    d_wkt = nc.dram_tensor("wkt", [128, 2, C], f8, kind="ExternalInput")
    d_sb = nc.dram_tensor("sb", [128, 2, 3], f32, kind="ExternalInput")
    d_ag = nc.dram_tensor("ag", [128, 2, G], f32, kind="ExternalInput")
    d_bg = nc.dram_tensor("bg", [G, 2, 128], f32, kind="ExternalInput")
    d_out = nc.dram_tensor("out", [128, 2, NH], f32, kind="ExternalOutput")

    with tile.TileContext(nc) as tc:
        with (
            tc.tile_pool(name="big", bufs=1) as big,
            tc.tile_pool(name="cst", bufs=1) as cst,
            tc.tile_pool(name="wrk", bufs=2) as wrk,
            tc.tile_pool(name="epool", bufs=3) as epool,
            tc.tile_pool(name="gnp", bufs=1) as gnp,
            tc.tile_pool(name="ps_s", bufs=2, space="PSUM") as ps_s,
            tc.tile_pool(name="ps_o", bufs=1, space="PSUM") as ps_o,
            tc.tile_pool(name="ps_r", bufs=1, space="PSUM") as ps_r,
            tc.tile_pool(name="ps_x", bufs=1, space="PSUM") as ps_x,
        ):
            # ---- input DMAs: x first (gates GN stats); big descriptors,
            # few triggers; params on scalar after x ----
            xf8 = big.tile([128, 2, HW], f8, tag="xf8")
            bstat = gnp.tile([128, 2, 8, 6], f32, tag="bstat")
            for ci in range(2):
                for h in range(2):
                    sl = slice(h * NH, (h + 1) * NH)
                    eng = nc.sync if h == 0 else nc.scalar
                    eng.dma_start(out=xf8[:, ci, sl], in_=d_xf.ap()[:, ci, sl])
            xt8 = big.tile([128, MB, C], f8, tag="xt8")
            for h in range(2):
                sl = slice(h * MB // 2, (h + 1) * MB // 2)
                eng = nc.sync if h == 0 else nc.scalar
                eng.dma_start(out=xt8[:, sl, :], in_=d_xt.ap()[:, sl, :])
            for ci in range(2):
                for j in range(8):
                    nc.vector.bn_stats(
                        out=bstat[:, ci, j, :],
                        in_=xf8[:, ci, j * 512:(j + 1) * 512])
            wkt8 = cst.tile([128, 2, C], f8, tag="wkt8")
            nc.scalar.dma_start(out=wkt8, in_=d_wkt.ap())
            smalls = cst.tile([128, 2, 3], f32, tag="smalls")
            nc.scalar.dma_start(out=smalls, in_=d_sb.ap())
            qb = smalls[:, :, 0:1]
            gb = smalls[:, :, 2:3]
            ag = cst.tile([128, 2, G], f32, tag="ag")
            nc.scalar.dma_start(out=ag, in_=d_ag.ap())
            bg = cst.tile([G, 2, 128], f32, tag="bg")
            nc.scalar.dma_start(out=bg, in_=d_bg.ap())
            wall = cst.tile([128, 2, 4 * C], bf16, tag="wall")
            nc.scalar.dma_start(out=wall, in_=d_w.ap())
            xo = big.tile([128, 2, NH], f32, tag="xo")
            for ci in range(2):
                nc.sync.dma_start(out=xo[:, ci, :], in_=d_xo.ap()[:, ci, :])

            epst = cst.tile([G, 1], f32, tag="epst")
            nc.vector.memset(epst, EPS)
            negc = cst.tile([128, 1], f32, tag="negc")  # softmax logit shift
            nc.vector.memset(negc, -3.0)
            # R lhsT (DoubleRow, M=128: R lands pre-broadcast on all rows)
            ones8 = cst.tile([128, 2, 128], f8, tag="ones8")
            nc.vector.memset(ones8, 1.0)

            # ---- GroupNorm stats aggregation ----
            stats2 = gnp.tile([128, 2, 2], f32, tag="stats2")  # (mean, E[x^2])
            tmp1 = gnp.tile([128, 1], f32, tag="tmp1")
            for ci in range(2):
                nc.vector.bn_aggr(out=stats2[:, ci, :], in_=bstat[:, ci, :, :])
                nc.vector.tensor_tensor(
                    out=tmp1, in0=stats2[:, ci, 0:1], in1=stats2[:, ci, 0:1],
                    op=OP.mult)
                nc.vector.tensor_tensor(
                    out=stats2[:, ci, 1:2], in0=stats2[:, ci, 1:2], in1=tmp1,
                    op=OP.add)
            # group sums across partitions: [G, 2] = sum_ci ag[ci]^T stats2[ci]
            pg = ps_x.tile([G, 2], f32, tag="x")
            for ci in range(2):
                nc.tensor.matmul(pg, lhsT=ag[:, ci, :], rhs=stats2[:, ci, :],
                                 start=(ci == 0), stop=(ci == 1))
            # ag carries 1/CPG so pg is directly (mean_g, E[x^2]_g)
            pgs = gnp.tile([G, 2], f32, tag="pgs")
            nc.vector.tensor_copy(out=pgs, in_=pg)
            gst = gnp.tile([G, 4], f32, tag="gst")  # mean^2, var, sd, -
            nc.vector.tensor_tensor(out=gst[:, 0:1], in0=pgs[:, 0:1],
                                    in1=pgs[:, 0:1], op=OP.mult)
            nc.vector.tensor_tensor(out=gst[:, 1:2], in0=pgs[:, 1:2],
                                    in1=gst[:, 0:1], op=OP.subtract)
            gfin = gnp.tile([G, 2], f32, tag="gfin")  # (rstd_g, mean_g*rstd_g)
            nc.scalar.activation(out=gst[:, 2:3], in_=gst[:, 1:2],
                                 func=AF.Sqrt, bias=epst)
            nc.vector.reciprocal(out=gfin[:, 0:1], in_=gst[:, 2:3])
            nc.vector.tensor_tensor(out=gfin[:, 1:2], in0=pgs[:, 0:1],
                                    in1=gfin[:, 0:1], op=OP.mult)
            # bg carries gn_w, so pbc = (scale_c, mean_c*scale_c);
            # bias_c = gn_b - mean_c*scale_c
            scbc = gnp.tile([128, 2, 2], f32, tag="scbc")
            for ci in range(2):
                pbc = ps_x.tile([128, 2], f32, tag="x")
                nc.tensor.matmul(pbc, lhsT=bg[:, ci, :], rhs=gfin,
                                 start=True, stop=True)
                nc.vector.tensor_copy(out=scbc[:, ci, 0:1], in_=pbc[:, 0:1])
                nc.vector.tensor_tensor(out=scbc[:, ci, 1:2], in0=gb[:, ci, :],
                                        in1=pbc[:, 1:2], op=OP.subtract)

            # ---- fp8 weights (gate q/qk production):
            # W8 = W * scale_c * 16 (q,v), W * 16 (proj)
            w8 = cst.tile([128, 2, 4 * C], f8, tag="w8")
            for ci in range(2):
                nc.vector.tensor_scalar(
                    out=w8[:, ci, 0:3 * C], in0=wall[:, ci, 0:3 * C],
                    scalar1=scbc[:, ci, 0:1], scalar2=SW,
                    op0=OP.mult, op1=OP.mult)
                nc.vector.tensor_scalar(
                    out=w8[:, ci, 3 * C:4 * C], in0=wall[:, ci, 3 * C:4 * C],
                    scalar1=SW, scalar2=None, op0=OP.mult)

            # ---- per-partition drain biases from the GN fold (bias_c on
            # unscaled bf16 W). The k-side bias shifts every logit of a query
            # equally -- softmax-invariant -- so it is dropped.
            bcr = gnp.tile([128, 2, 2], bf16, tag="bcr")  # bias_c, 2 copies
            for ci in range(2):
                nc.vector.tensor_copy(out=bcr[:, ci, 0:1],
                                      in_=scbc[:, ci, 1:2])
                nc.vector.tensor_copy(out=bcr[:, ci, 1:2],
                                      in_=scbc[:, ci, 1:2])
            # bias2[:, cb, 0] = qb + Wq @ bias_c ; bias2[:, cb, 1] = Wv @ bias_c
            bias2 = gnp.tile([128, 2, 2], f32, tag="bias2")
            for wi, woff in ((0, 0), (1, 2 * C)):
                for cb in range(2):
                    pbias = ps_x.tile([128, 2], f32, tag="x")
                    for ci in range(2):
                        nc.tensor.matmul(
                            pbias,
                            lhsT=wall[:, ci, woff + cb * 128:
                                      woff + (cb + 1) * 128],
                            rhs=bcr[:, ci, :], start=(ci == 0), stop=(ci == 1))
                    if wi == 0:
                        nc.vector.tensor_tensor(
                            out=bias2[:, cb, 0:1], in0=pbias[:, 0:1],
                            in1=qb[:, cb, :], op=OP.add)
                    else:
                        nc.vector.tensor_copy(out=bias2[:, cb, 1:2],
                                              in_=pbias[:, 0:1])

            qt = big.tile([128, 2, NH], f8, tag="qt")
            qk = big.tile([128, 2, NH], f8, tag="qk")

            def qt_unit(j):  # q for 512 queries: Wq' x + bq
                sl = slice(j * 512, (j + 1) * 512)
                pq = ps_s.tile([128, 2, 512], f32, tag="s", name=f"pq{j}")
                for cb in range(2):
                    nc.tensor.matmul(
                        pq[:, cb, :],
                        lhsT=w8[:, :, cb * 128:(cb + 1) * 128],
                        rhs=xf8[:, :, sl], start=True, stop=True,
                        perf_mode=PM.DoubleRow)
                for cb in range(2):
                    nc.vector.tensor_scalar(
                        out=qt[:, cb, sl], in0=pq[:, cb, :],
                        scalar1=1.0 / SW, scalar2=bias2[:, cb, 0:1],
                        op0=OP.mult, op1=OP.add)

            def qk_unit(j):  # qk = scale_c * (Wk^T q) for 512 queries
                sl = slice(j * 512, (j + 1) * 512)
                pk = ps_s.tile([128, 2, 512], f32, tag="s", name=f"pqk{j}")
                for ci in range(2):
                    nc.tensor.matmul(
                        pk[:, ci, :],
                        lhsT=wkt8[:, :, ci * 128:(ci + 1) * 128],
                        rhs=qt[:, :, sl], start=True, stop=True,
                        perf_mode=PM.DoubleRow)
                for ci in range(2):
                    nc.vector.tensor_scalar(
                        out=qk[:, ci, sl], in0=pk[:, ci, :],
                        scalar1=scbc[:, ci, 0:1], scalar2=1.0 / SW,
                        op0=OP.mult, op1=OP.mult)

            state = {}

            def s_pair(j, p):
                sl = slice(j * 512, (j + 1) * 512)
                sp = ps_s.tile([128, 2, 512], f32, tag="s", name=f"sp{j}_{p}")
                for par in range(2):
                    mb = 2 * p + par
                    nc.tensor.matmul(
                        sp[:, par, :],
                        lhsT=xf8[:, :, mb * 128:(mb + 1) * 128],
                        rhs=qk[:, :, sl], start=True, stop=True,
                        perf_mode=PM.DoubleRow)
                return sp

            def xe_r(j, p, et):
                po, pr = state["po"], state["pr"]
                for ci in range(2):
                    nc.tensor.matmul(
                        po[:, ci, :],
                        lhsT=xt8[:, 2 * p:2 * p + 2, ci * 128:(ci + 1) * 128],
                        rhs=et, start=(p == 0), stop=(p == NP - 1),
                        perf_mode=PM.DoubleRow, skip_group_check=True)
                nc.tensor.matmul(
                    pr, lhsT=ones8, rhs=et,
                    start=(p == 0), stop=(p == NP - 1),
                    perf_mode=PM.DoubleRow, skip_group_check=True)

            def attn_chunk(j, inject=None, produce=None):
                state["po"] = ps_o.tile([128, 2, 512], f32, tag="o",
                                        name=f"po{j}")
                state["pr"] = ps_r.tile([128, 512], f32, tag="r",
                                        name=f"pr{j}")
                prev_et = None
                for p in range(NP):
                    sp = s_pair(j, p)
                    if prev_et is not None:
                        xe_r(j, p - 1, prev_et)
                    if inject is not None and 2 <= p <= 5:
                        tail_piece(inject, p - 2)
                    if p == 7 and produce is not None:
                        qt_unit(produce)
                    if p == 11 and produce is not None:
                        qk_unit(produce)
                    # logit shift keeps exp in e4m3 range (max logit ~8:
                    # e^(8-3)=148 < 240); softmax is shift-invariant
                    et = epool.tile([128, 2, 512], f8, tag="et")
                    nc.scalar.activation(out=et, in_=sp, func=AF.Exp,
                                         scale=1.0 / SW, bias=negc)
                    prev_et = et
                xe_r(j, NP - 1, prev_et)

            def tail_norm(j, last=False):
                """Free po/pr: normalized xE in fp8, 1/R via fast recip."""
                po, pr = state["po"], state["pr"]
                rb = wrk.tile([128, 512], f32, tag="rb")
                nc.vector.reciprocal_approx_fast(out=rb, in_=pr)  # frees pr
                xen = wrk.tile([128, 2, 512], f8, tag="xen")
                if last:
                    # nothing follows: skip the po-freeing copy
                    for ci in range(2):
                        nc.vector.tensor_tensor(
                            out=xen[:, ci, :], in0=po[:, ci, :], in1=rb,
                            op=OP.mult)
                else:
                    onorm = wrk.tile([128, 2, 512], f32, tag="onorm")
                    nc.vector.tensor_copy(out=onorm, in_=po)  # frees po
                    for ci in range(2):
                        nc.vector.tensor_tensor(
                            out=xen[:, ci, :], in0=onorm[:, ci, :], in1=rb,
                            op=OP.mult)
                state[f"xen{j}"] = xen

            def tail_piece(j, step):
                """One step of: attn = Wv xEn + bv, out = Wp attn/16 + xo.
                Split into 4 single-matmul pieces so the lone ps_x bank's
                drain latency hides under the pair cadence."""
                sl = slice(j * 512, (j + 1) * 512)
                if step == 0:
                    state[f"attn{j}"] = wrk.tile([128, 2, 512], f8,
                                                 tag="attn8",
                                                 name=f"attn{j}")
                if step < 2:
                    cb = step
                    xen = state[f"xen{j}"]
                    pa = ps_x.tile([128, 512], f32, tag="x", name=f"pa{j}_{cb}")
                    nc.tensor.matmul(
                        pa, lhsT=w8[:, :, 2 * C + cb * 128:
                                    2 * C + (cb + 1) * 128],
                        rhs=xen, start=True, stop=True,
                        perf_mode=PM.DoubleRow)
                    nc.vector.tensor_scalar(
                        out=state[f"attn{j}"][:, cb, :], in0=pa,
                        scalar1=1.0 / SW, scalar2=bias2[:, cb, 1:2],
                        op0=OP.mult, op1=OP.add)
                else:
                    co = step - 2
                    pp = ps_x.tile([128, 512], f32, tag="x", name=f"pp{j}_{co}")
                    nc.tensor.matmul(
                        pp, lhsT=w8[:, :, 3 * C + co * 128:
                                    3 * C + (co + 1) * 128],
                        rhs=state[f"attn{j}"], start=True, stop=True,
                        perf_mode=PM.DoubleRow)
                    outt = wrk.tile([128, 512], f32, tag="outt")
                    nc.vector.scalar_tensor_tensor(
                        out=outt, in0=pp, scalar=1.0 / SW,
                        in1=xo[:, co, sl], op0=OP.mult, op1=OP.add)
                    nc.sync.dma_start(out=d_out.ap()[:, co, sl], in_=outt)
                    if co == 1:
                        state.pop(f"xen{j}")
                        state.pop(f"attn{j}")

            def tail_proj(j):
                for step in range(4):
                    tail_piece(j, step)

            # ---- chunks; q/qk for chunk j+1 are produced inside chunk j ----
            qt_unit(0)
            qk_unit(0)
            attn_chunk(0, produce=1)
            tail_norm(0)
            attn_chunk(1, inject=0, produce=2)
            tail_norm(1)
            attn_chunk(2, inject=1, produce=3)
            tail_norm(2)
            attn_chunk(3, inject=2)
            tail_norm(3, last=True)
            tail_proj(3)

    nc.compile()
    _cache["nc"] = nc
    return nc


def _prep_maps(x, gn_w, gn_b, qkv_w, qkv_b, proj_w, proj_b):
    """Host-side sharding + layout prep. Returns list of 8 in_maps."""
    import ml_dtypes
    f8 = ml_dtypes.float8_e4m3
    bf16 = ml_dtypes.bfloat16

    x = np.asarray(x, np.float32)
    qkv_w = np.asarray(qkv_w, np.float32)
    qkv_b = np.asarray(qkv_b, np.float32)
    proj_w = np.asarray(proj_w, np.float32)
    proj_b = np.asarray(proj_b, np.float32)
    gn_w = np.asarray(gn_w, np.float32)
    gn_b = np.asarray(gn_b, np.float32)

    def chunked(a):  # [256, ...] -> [128, 2, ...]
        return np.ascontiguousarray(a.reshape(2, 128, *a.shape[1:]).transpose(
            1, 0, *range(2, a.ndim + 1)))

    wq = qkv_w[0:C].T
    wk = qkv_w[C:2 * C].T
    wv = qkv_w[2 * C:3 * C].T
    wp = proj_w.T
    wall = chunked(np.concatenate([wq, wk, wv, wp], axis=1)).astype(bf16)
    # wkt: Wk as [c_out, c_in], raw (no GN scale), pre-scaled by 16 for fp8
    wkt = chunked(qkv_w[C:2 * C] * SW).astype(f8)
    rbias = proj_w @ qkv_b[2 * C:3 * C] + proj_b   # v-bias fold + proj bias
    smalls = chunked(np.stack([qkv_b[0:C], qkv_b[C:2 * C], gn_b], axis=1))

    cidx = np.arange(C)
    ag_full = (cidx[:, None] // CPG == np.arange(G)[None, :]).astype(np.float32)
    ag = chunked(ag_full / CPG)                     # [128, 2, G], carries 1/8
    bg_full = ag_full * gn_w[:, None]               # carries gn_w
    bg = np.ascontiguousarray(
        bg_full.reshape(2, 128, G).transpose(2, 0, 1))  # [G, 2, 128]

    maps = []
    for core in range(8):
        b, half = core // 2, core % 2
        xflat = x[b].reshape(C, HW)
        own = xflat[:, half * NH:(half + 1) * NH]
        other = xflat[:, (1 - half) * NH:(2 - half) * NH]
        xroll = np.concatenate([own, other], axis=1)  # own queries first
        xt = np.ascontiguousarray(
            xroll.T.reshape(MB, 128, C).transpose(1, 0, 2))
        xo = own + rbias[:, None]
        maps.append({
            "xf": chunked(xroll).astype(f8),
            "xt": xt.astype(f8),
            "xo": chunked(xo),
            "w": wall, "wkt": wkt, "sb": smalls, "ag": ag, "bg": bg,
        })
    return maps


def kernel(x, gn_w, gn_b, qkv_w, qkv_b, proj_w, proj_b):
    import concourse.bass_utils as bu
    nc = build_nc()
    maps = _prep_maps(x, gn_w, gn_b, qkv_w, qkv_b, proj_w, proj_b)
    res = bu.run_bass_kernel_spmd(nc, maps, core_ids=list(range(8)))
    out = np.empty((B, C, HW), np.float32)
    for core in range(8):
        b, half = core // 2, core % 2
        o = res.results[core]["out"]                # [128, 2, NH]
        out[b, :, half * NH:(half + 1) * NH] = \
            o.transpose(1, 0, 2).reshape(C, NH)
    return out.reshape(B, C, 64, 64)
